# revision 14
# baseline (speedup 1.0000x reference)
"""Device-pure Fourier-domain kernel for nn_EquiLinearRegToReg, v3.

The operator is block-circulant over the k-axis; DFT diagonalization gives
three matmul stages.  All intermediates stay in SBUF; the two partition
re-layouts (DFT output planes -> contraction rows, matmul rows -> iDFT
rows) are SBUF->SBUF DMAs.  Relayout destinations use *interleaved*
(i8-major / j8-major) partition maps so each DMA's 8 destination
partitions are spread stride-16 across the partition space - this spreads
descriptors over many SDMA engines (contiguous-8 destinations collapse
onto one or two port groups and serialize).

S1: f_hat = DFT_x(field), 32 matmuls with a block-diagonal stationary
    (partition (x,i8) -> (i8,q)); shuffled into fht rows i8*16 + ig%16.
S2: per-frequency complex matmuls, 120 matmuls; weight slabs dedup'd to
    46 (Wr shared between re/im output planes); PSUM columns ordered
    (j8, jg) so each shuffle into oht rows j8*16 + plane is a plain
    [128, bp] -> [8(stride 16), 16, bp] DMA.
S3: iDFT via block-diagonal stationary (partition (j8,p) -> (j8,y)),
    32 matmuls, f16 output written partition-major (host casts to f32).

PSUM is evicted in 4-bank quads ([128, 2048] f32 -> f16) alternating
Vector/Scalar.  HBM traffic per core: field 4MB + slabs 2.9MB + out 4MB.

Plane order: [w0, re1, im1, ..., re7, im7, w8] (16 real planes).
"""

import os
import numpy as np

import concourse.mybir as mybir
import concourse.tile as tile
from concourse import bacc
from concourse.bass_utils import run_bass_kernel_spmd

BATCH, NUM_PART, IN_FEAT, OUT_FEAT, K = 8, 512, 256, 256, 16
N_CORES = 8
P = 128
IO = IN_FEAT // P           # 2 i-slabs of 128
NIG = IN_FEAT // 8          # 32 i-groups of 8
NJG = OUT_FEAT // 8         # 32 j-groups of 8
JC = OUT_FEAT // P          # 2 j-chunks of 128
NSLAB = 46
NW2P = 2                    # weight slab load pieces

_CACHE = {}

PLANES = [(0, "re")] + [(w, k) for w in range(1, 8) for k in ("re", "im")] + [(8, "re")]


def _slab_base(w):
    return 4 + (w - 1) * 6


def _s2_ktiles(pp):
    """For out-plane pp: list of (f_hat plane q, slab sid); io = sid % 2."""
    w, kind = PLANES[pp]
    if w == 0:
        return [(pp, 0), (pp, 1)]
    if w == 8:
        return [(pp, 2), (pp, 3)]
    b = _slab_base(w)
    if kind == "re":   # Hr = Fr Wr + Fi (-Wi)
        return [(pp, b), (pp, b + 1), (pp + 1, b + 4), (pp + 1, b + 5)]
    else:              # Hi = Fr Wi + Fi Wr
        return [(pp - 1, b + 2), (pp - 1, b + 3), (pp, b), (pp, b + 1)]


def _cf():
    C = np.zeros((K, K))
    x = np.arange(K)
    for p, (w, kind) in enumerate(PLANES):
        C[:, p] = np.cos(2 * np.pi * w * x / K) if kind == "re" else -np.sin(2 * np.pi * w * x / K)
    return C


def _ci():
    C = np.zeros((K, K))
    y = np.arange(K)
    for p, (w, kind) in enumerate(PLANES):
        s = 1.0 / K if w in (0, 8) else 2.0 / K
        C[p, :] = s * np.cos(2 * np.pi * w * y / K) if kind == "re" else -s * np.sin(2 * np.pi * w * y / K)
    return C


def _w2p_bounds():
    edges = np.linspace(0, NSLAB, NW2P + 1).astype(int)
    return [(int(edges[i]), int(edges[i + 1])) for i in range(NW2P)]


def _build():
    if "nc" in _CACHE:
        return _CACHE["nc"]
    f32 = mybir.dt.float32
    f16 = mybir.dt.float16

    nc = bacc.Bacc(None, target_bir_lowering=False, debug=False)
    fieldx_d = nc.dram_tensor("fieldx", [P, NIG, NUM_PART], f16, kind="ExternalInput")
    b13_d = nc.dram_tensor("b13", [P, 2, P], f16, kind="ExternalInput")
    w2_d = nc.dram_tensor("w2", [P, NSLAB, OUT_FEAT], f16, kind="ExternalInput")
    out_d = nc.dram_tensor("out", [P, NJG, NUM_PART], f16, kind="ExternalOutput")

    bounds = _w2p_bounds()

    with tile.TileContext(nc) as tc:
        with (
            tc.tile_pool(name="const", bufs=1) as const,
            tc.tile_pool(name="sb", bufs=2) as sb,
            tc.tile_pool(name="st", bufs=4) as st,
            tc.tile_pool(name="psum", bufs=2, space="PSUM") as psum,
        ):
            zbias = const.tile([P, 1], f32, name="zbias", tag="zb", bufs=1)
            nc.vector.memset(zbias[:], 0.0)

            b13 = const.tile([P, 2, P], f16, name="b13", tag="b13", bufs=1)
            b1, b3 = b13[:, 0, :], b13[:, 1, :]
            fht = const.tile([P, IO, K, NUM_PART], f16, name="fht", tag="fh", bufs=1)
            ohts = [
                const.tile([P, K, NUM_PART], f16, name=f"oht{jc}", tag=f"oh{jc}", bufs=1)
                for jc in range(JC)
            ]
            w2ps = [
                const.tile([P, s1 - s0, OUT_FEAT], f16, name=f"w2p{i}",
                           tag=f"w2p{i}", bufs=1)
                for i, (s0, s1) in enumerate(bounds)
            ]
            # port-interleaved relayout destinations: each shuffle's 8
            # destination partitions sit at stride 4 inside one 32-row
            # block, touching 8 of the 16 SBUF ports (stride-16 rows only
            # touch 4, which caps SDMA concurrency per op).
            # fht row = (g//4)*32 + i8*4 + g%4 for i_global io*128+g*8+i8
            fhv = fht[:].rearrange("(gh i8 gl) io q bp -> gh gl i8 io q bp",
                                   gh=4, i8=8)
            # oht row = (pp//4)*32 + j8*4 + pp%4
            ohvs = [ohts[jc][:].rearrange("(ph j8 pl) jg bp -> ph pl j8 jg bp",
                                          ph=4, j8=8) for jc in range(JC)]

            def slab(sid, jc):
                for i, (s0, s1) in enumerate(bounds):
                    if s0 <= sid < s1:
                        return w2ps[i][:, sid - s0, jc * P:(jc + 1) * P]
                raise AssertionError(sid)

            # HBM inputs: field quads pipelined early on the sync/SP HWDGE
            # ring (16 engines, low latency); weight slabs on the gpsimd
            # SWDGE ring (its ~7us Q7 emission latency is harmless there).
            # All shuffles + outputs also ride the sync ring, whose
            # descriptors spread across all 16 SDMA engines.
            # asymmetric field tiles (2/2/4/8/16 i-groups): tiny tiles up
            # front so S1 starts early, big descriptors for the bulk; the
            # 1MB tile rides the ACT ring to parallelize the HBM reads.
            fxw = [2, 2, 4, 8, 16]
            fx_eng = [nc.sync, nc.sync, nc.sync, nc.scalar, nc.sync]
            fxt, ig_src = [], {}
            igoff = 0
            for ti, wig in enumerate(fxw):
                fx = sb.tile([P, wig, NUM_PART], f16, tag=f"fx{ti}",
                             bufs=1, name=f"fx{ti}")
                fx_eng[ti].dma_start(
                    fx[:], fieldx_d[:, igoff:igoff + wig, :])
                fxt.append(fx)
                for dig in range(wig):
                    ig_src[igoff + dig] = (ti, dig)
                igoff += wig
                if ti == 0:
                    nc.sync.dma_start(b13[:], b13_d[:])
            for i, (s0, s1) in enumerate(bounds):
                nc.gpsimd.dma_start(w2ps[i][:], w2_d[:, s0:s1, :])

            shuf_n = 0

            def shuffle(dst, src):
                nonlocal shuf_n
                eng = (nc.sync, nc.scalar, nc.gpsimd,
                       nc.sync if (shuf_n // 4) % 2 == 0 else nc.gpsimd)[shuf_n % 4]
                shuf_n += 1
                eng.dma_start(dst, src)

            evict_n = 0

            def evict(dst, acc):
                """Quad eviction PSUM f32 -> SBUF f16, alternating DVE/ACT."""
                nonlocal evict_n
                evict_n += 1
                if evict_n % 3 != 2:
                    nc.vector.tensor_copy(dst, acc)
                else:
                    nc.scalar.activation(
                        dst, acc, mybir.ActivationFunctionType.Identity,
                        bias=zbias[:])

            # ---- S1 ----  (quad = 4 i-groups; shuffles on the sync ring)
            for b in range(NIG // 4):
                acc = psum.tile([P, 4 * NUM_PART], f32, tag="ps", name=f"s1p{b}")
                for k4 in range(4):
                    ti, dig = ig_src[b * 4 + k4]
                    nc.tensor.matmul(acc[:, k4 * NUM_PART:(k4 + 1) * NUM_PART],
                                     b1, fxt[ti][:, dig, :],
                                     start=True, stop=True)
                sg = st.tile([P, 4, NUM_PART], f16, tag="sg", bufs=4,
                             name=f"sg{b}")
                evict(sg[:].rearrange("p a n -> p (a n)"), acc[:])
                io = b // 4
                for k4 in range(4):
                    g = (b % 4) * 4 + k4
                    shuffle(fhv[g // 4, g % 4, :, io], sg[:, k4, :])

            # keep PE warm across the weight-load gate (HAM re-throttles
            # after ~3.4us of PE idle; a throttled restart costs ~2x for
            # its first microseconds)
            warm = psum.tile([P, 4 * NUM_PART], f32, tag="ps", name="warm")
            for _ in range(6):
                nc.tensor.matmul(warm[:, 0:NUM_PART], b1, fxt[4][:, 15, :],
                                 start=True, stop=True)

            # ---- S2 ----  (quad = 4 output planes; shuffles on sync)
            for jc in range(JC):
                for t in range(4):
                    acc = psum.tile([P, 4 * NUM_PART], f32, tag="ps",
                                    name=f"s2p{jc}_{t}")
                    for k4 in range(4):
                        pp = t * 4 + k4
                        kts = _s2_ktiles(pp)
                        for ki, (q, sid) in enumerate(kts):
                            nc.tensor.matmul(
                                acc[:, k4 * NUM_PART:(k4 + 1) * NUM_PART],
                                slab(sid, jc),
                                fht[:, sid % 2, q, :],
                                start=(ki == 0),
                                stop=(ki == len(kts) - 1),
                            )
                    hg = st.tile([P, 4, NUM_PART], f16, tag="hg", bufs=4,
                                 name=f"hg{jc}_{t}")
                    evict(hg[:].rearrange("p a n -> p (a n)"), acc[:])
                    for k4 in range(4):
                        pp = t * 4 + k4
                        shuffle(ohvs[jc][pp // 4, pp % 4], hg[:, k4, :])

            # ---- S3 ----  (quad = 4 j-groups; 8-group output DMAs on the
            # ACT ring, which is idle by then)
            for jc in range(JC):
                for g2 in range(2):
                    og = st.tile([P, 8, NUM_PART], f16, tag="og", bufs=2,
                                 name=f"og{jc}_{g2}")
                    for qd in range(2):
                        acc = psum.tile([P, 4 * NUM_PART], f32, tag="ps",
                                        name=f"s3p{jc}_{g2}_{qd}")
                        for k4 in range(4):
                            jgl = g2 * 8 + qd * 4 + k4
                            nc.tensor.matmul(
                                acc[:, k4 * NUM_PART:(k4 + 1) * NUM_PART],
                                b3, ohts[jc][:, jgl, :],
                                start=True, stop=True)
                        evict(og[:, qd * 4:(qd + 1) * 4, :]
                              .rearrange("p a n -> p (a n)"), acc[:])
                    jg0 = jc * 16 + g2 * 8
                    nc.gpsimd.dma_start(out_d[:, jg0:jg0 + 8, :], og[:])

    nc.compile()
    _CACHE["nc"] = nc
    return nc


def _prep_inputs(field_feat, weights):
    field_feat = np.ascontiguousarray(field_feat, dtype=np.float32)
    weights = np.ascontiguousarray(weights, dtype=np.float32)

    Cf, Ci = _cf(), _ci()
    B1 = np.zeros((P, P), np.float32)
    for x in range(K):
        for i8 in range(8):
            B1[x * 8 + i8, i8 * 16:(i8 + 1) * 16] = Cf[x]
    # oht partition r = (p//4)*32 + j8*4 + p%4; out partition = j8*16 + y
    B3 = np.zeros((P, P), np.float32)
    for r in range(P):
        p = (r // 32) * 4 + r % 4
        j8 = (r % 32) // 4
        B3[r, j8 * 16:(j8 + 1) * 16] = Ci[p]
    Wf = np.fft.fft(weights, axis=2)

    # fht row r (per io half) holds i_global = io*128 + g*8 + i8 with
    # g = (r//32)*4 + r%4, i8 = (r%32)//4
    iperm = np.empty(P, np.int64)
    for r in range(P):
        iperm[r] = ((r // 32) * 4 + r % 4) * 8 + (r % 32) // 4
    # S2 PSUM column order (j8 major, jg minor): col j8*16+jgl <-> j = jgl*8+j8
    jperm = np.empty(OUT_FEAT, np.int64)
    for jc in range(JC):
        for j8 in range(8):
            for jgl in range(16):
                jperm[jc * P + j8 * 16 + jgl] = jc * P + jgl * 8 + j8

    W2U = np.zeros((P, NSLAB, OUT_FEAT), np.float32)

    def put(sid, S):
        Sp = S[:, jperm]
        W2U[:, sid, :] = Sp[iperm, :]
        W2U[:, sid + 1, :] = Sp[P + iperm, :]

    put(0, Wf[:, :, 0].real.astype(np.float32))
    put(2, Wf[:, :, 8].real.astype(np.float32))
    for w in range(1, 8):
        b = _slab_base(w)
        Wr = Wf[:, :, w].real.astype(np.float32)
        Wi = Wf[:, :, w].imag.astype(np.float32)
        put(b, Wr)
        put(b + 2, Wi)
        put(b + 4, -Wi)
    w2 = np.ascontiguousarray(W2U, dtype=np.float16)

    in_maps = []
    b13 = np.ascontiguousarray(
        np.stack([B1, B3], axis=1), dtype=np.float16)
    for c in range(N_CORES):
        f = field_feat[c].transpose(1, 2, 0)                  # [i, x, bp]
        fx = f.reshape(NIG, 8, K, NUM_PART).transpose(2, 1, 0, 3)
        fx = np.ascontiguousarray(fx.reshape(P, NIG, NUM_PART),
                                  dtype=np.float16)
        in_maps.append({"fieldx": fx, "b13": b13, "w2": w2})
    return in_maps


def kernel(field_feat, weights):
    nc = _build()
    in_maps = _prep_inputs(field_feat, weights)
    trace = bool(int(os.environ.get("KERNEL_TRACE", "0")))
    # NRT occasionally reports a transient EXEC_UNIT_UNRECOVERABLE on the
    # first execute of a fresh session; a retry on a new session passes.
    for attempt in range(3):
        try:
            res = run_bass_kernel_spmd(nc, in_maps, list(range(N_CORES)),
                                       trace=trace)
            break
        except Exception:  # noqa: BLE001
            if attempt == 2:
                raise
    if trace:
        kernel.last_exec_time_ns = res.exec_time_ns
        kernel.last_results = res
    # out[j8*16+y, jg, bp] -> [bp, j, y]
    outs = []
    for c in range(N_CORES):
        o = res.results[c]["out"].reshape(8, K, NJG, NUM_PART)
        outs.append(o.transpose(3, 2, 0, 1).reshape(NUM_PART, OUT_FEAT, K))
    return np.stack(outs).reshape(BATCH, NUM_PART, OUT_FEAT, K).astype(np.float32)
